# revision 55
# baseline (speedup 1.0000x reference)
"""Trainium2 Bass kernel for the CNNFusing ragged-session attention pooling module.

Computes, per session s over its token set:
    v_mean   = mean(hidden[s])                                  [H]
    ph[t]    = tanh(hidden[t] @ Wp1 + (pos_table @ Wp2 + b_pos)[rp[t]])
    gate[t]  = sigmoid(v_mean @ W1 + b1 + ph[t] @ W2 + b2)
    alpha[t] = gate[t] @ qw + qb
    h_s      = sum_t alpha[t] * hidden[t]                       [B, H]

Data-parallel over sessions on 8 cores.  Sessions are FFD-bin-packed into
512-token chunks (<=32 sessions per chunk).  Ragged ops become one-hot
matmuls.  The two big H x H GEMM chains per token (ph pre-act and the gate
ph-term) run as fp8e4m3 DoubleRow matmuls (2 contraction rows/partition);
session sums / weighted sums stay fp16.  h_s is accumulated transposed
([h, s] layout) so each accumulation step costs S output rows instead of H.
The sigmoid is folded into tanh and both per-token biases ride the one-hot
mean-term matmul, so each stage needs a single full-width tanh activation.
"""

import numpy as np
import ml_dtypes

import concourse.bass as bass
import concourse.mybir as mybir
import concourse.tile as tile
from concourse.bass_utils import run_bass_kernel_spmd

H = 256
TC = 512      # tokens per chunk
S = 32        # max sessions per chunk
KT = TC // 128
G = 2         # chunks per batched load DMA
SG = 8      # chunks per seg-row broadcast DMA
GST = 8     # chunks per batched store
N_CORES = 8

F8 = mybir.dt.float8e4
F16 = mybir.dt.float16
F32 = mybir.dt.float32
NP_F8 = ml_dtypes.float8_e4m3fn


# --------------------------------------------------------------------------
# The walrus build here accepts only ONE sync-wait command per instruction,
# while Tile may attach several.  Hoist all but the last wait of such
# instructions onto standalone event-semaphore waits inserted just before
# them on the same engine (sequencer executes in order, semantics kept).
_waitsplit_uid = [0]


def _split_multi_waits(nc):
    for fn in nc.m.functions:
        for bb in fn.blocks:
            insts = bb.instructions
            i = 0
            while i < len(insts):
                inst = insts[i]
                si = getattr(inst, "sync_info", None)
                waits = list(si.on_wait) if si is not None and si.on_wait else []
                if len(waits) > 1:
                    si.on_wait = waits[-1:]
                    for w in waits[:-1]:
                        ev = mybir.InstEventSemaphore(
                            name=f"I-waitsplit-{_waitsplit_uid[0]}", ins=[], outs=[]
                        )
                        _waitsplit_uid[0] += 1
                        ev.engine = inst.engine
                        ev.sync_info = mybir.SyncInfo(on_wait=[w], on_update=[])
                        insts.insert(i, ev)
                        i += 1
                i += 1
# --------------------------------------------------------------------------


def _plan(seq_len):
    """Best-fit-decreasing bin packing of all sessions into (token<=TC,
    sessions<=S) chunks, then deal chunks round-robin to cores."""
    lens = np.asarray(seq_len, dtype=np.int64)
    B = len(lens)
    order = np.argsort(-lens, kind="stable")
    bins = []          # list of [tok_used, [session ids]]
    # rem_sorted: sorted list of (remaining_tokens, bin_idx) for best-fit
    import bisect
    rem = []           # sorted (remaining, bin_idx)
    for sid in order:
        L = int(lens[sid])
        # best fit: smallest remaining >= L
        pos = bisect.bisect_left(rem, (L, -1))
        placed = False
        while pos < len(rem):
            r, bi = rem[pos]
            if len(bins[bi][1]) < S:
                rem.pop(pos)
                bins[bi][0] += L
                bins[bi][1].append(sid)
                nr = TC - bins[bi][0]
                if nr > 0:
                    bisect.insort(rem, (nr, bi))
                placed = True
                break
            pos += 1
        if not placed:
            bi = len(bins)
            bins.append([L, [sid]])
            bisect.insort(rem, (TC - L, bi))
    nb = len(bins)
    C = -(-nb // N_CORES)
    core_chunks = [[] for _ in range(N_CORES)]
    for i, b in enumerate(bins):
        core_chunks[i % N_CORES].append(b[1])
    return lens, core_chunks, C


def _pack_inputs(hidden, reverse_pos, pw8, lens, core_chunks, C):
    """Build all per-core DRAM input arrays."""
    B = len(lens)
    starts = np.concatenate([[0], np.cumsum(lens)[:-1]])
    hidden16 = np.asarray(hidden, np.float32).astype(np.float16)
    rp = np.asarray(reverse_pos)

    # token map [N, C, TC] -> global token index (or -1)
    tokmap = np.full((N_CORES, C, TC), -1, np.int64)
    seg_row = np.full((N_CORES, C, TC), -1.0, np.float16)
    recip = np.zeros((N_CORES, S, C), np.float32)
    out_core = np.zeros(B, np.int32)
    out_chunk = np.zeros(B, np.int32)
    out_local = np.zeros(B, np.int32)

    for core in range(N_CORES):
        for ci, sess in enumerate(core_chunks[core]):
            t = 0
            for si, sid in enumerate(sess):
                L = int(lens[sid])
                tokmap[core, ci, t : t + L] = np.arange(starts[sid], starts[sid] + L)
                seg_row[core, ci, t : t + L] = si
                recip[core, si, ci] = 1.0 / L
                out_core[sid] = core
                out_chunk[sid] = ci
                out_local[sid] = si
                t += L

    valid = tokmap >= 0
    idx = np.where(valid, tokmap, 0)

    # gathered hidden [N, C, TC, H] fp16 (zero padded)
    xt = hidden16[idx]
    xt[~valid] = 0
    # row tiles [N, C, 128, KT, H] f16
    x16 = np.ascontiguousarray(
        xt.reshape(N_CORES, C, KT, 128, H).transpose(0, 1, 3, 2, 4)
    )
    # transposed fp8 [N, C, 128, 2, TC]
    xt8 = np.ascontiguousarray(
        xt.astype(NP_F8).transpose(0, 1, 3, 2).reshape(N_CORES, C, 2, 128, TC)
        .transpose(0, 1, 3, 2, 4)
    )
    del xt

    # pos features (already fp8-quantized table), gathered transposed
    rpg = np.where(valid, rp[idx], 0)
    pft = pw8[rpg]                                  # [N, C, TC, H] fp8
    pft[~valid] = 0
    pf8 = np.ascontiguousarray(
        pft.transpose(0, 1, 3, 2).reshape(N_CORES, C, 2, 128, TC)
        .transpose(0, 1, 3, 2, 4)
    )
    del pft

    seg_col = np.ascontiguousarray(
        seg_row.reshape(N_CORES, C, KT, 128).transpose(0, 3, 1, 2)
    ).astype(np.float32)

    return x16, xt8, pf8, seg_row, seg_col, recip, (out_core, out_chunk, out_local)


def _pack_weights(pos_table, W_pos, b_pos, W1, b1, W2, b2, qw, qb):
    Wp = np.asarray(W_pos, np.float32)
    pwf = np.asarray(pos_table, np.float32) @ Wp[H:] + np.asarray(b_pos, np.float32)
    pw8 = np.zeros((H, H), NP_F8)
    pw8[: pwf.shape[0]] = pwf.astype(NP_F8)

    def pack_dr(M):  # [256, 256] -> [128, 2, 256] fp8, row c = 128*i + p
        return np.ascontiguousarray(
            np.asarray(M, np.float32).reshape(2, 128, H).transpose(1, 0, 2)
        ).astype(NP_F8)

    wp18 = pack_dr(Wp[:H])
    w28 = pack_dr(np.asarray(W2, np.float32))
    w18 = pack_dr(np.asarray(W1, np.float32))

    ident8 = np.zeros((128, 2, H), NP_F8)
    for m in range(2):
        ident8[:, m, m * 128 : (m + 1) * 128] = np.eye(128, dtype=NP_F8)

    qwf = np.asarray(qw, np.float32).reshape(H)
    qwh = np.ascontiguousarray(qwf.reshape(2, 128).T).astype(np.float16)
    qbp = float(np.asarray(qb, np.float32).reshape(()) + qwf.sum() / 2.0)
    # full (unscaled) bias b1+b2 rides the mean-term; ACT applies tanh(z/2)
    bcf = np.asarray(b1, np.float32) + np.asarray(b2, np.float32)
    bchrow = np.broadcast_to(bcf, (S, H)).copy().astype(np.float32)

    iota_at = np.broadcast_to(np.arange(S, dtype=np.float16), (128, S)).copy()
    wk8 = np.concatenate([wp18, ident8, w28, w18], axis=2)
    cf16 = np.concatenate([qwh, iota_at], axis=1).astype(np.float16)
    return dict(wk8=wk8, cf16=cf16, bchrow=bchrow), qbp, pw8


def _build_bass(C, qbp):
    nc = bass.Bass("TRN2", target_bir_lowering=False, debug=False,
                   num_devices=N_CORES)

    x16 = nc.dram_tensor("x16", [C, 128, KT, H], F16, kind="ExternalInput")
    xt8 = nc.dram_tensor("xt8", [C, 128, 2, TC], F8, kind="ExternalInput")
    pf8 = nc.dram_tensor("pf8", [C, 128, 2, TC], F8, kind="ExternalInput")
    seg_row = nc.dram_tensor("seg_row", [C, TC], F16, kind="ExternalInput")
    W32 = C * KT + H + 1 + C
    wk8 = nc.dram_tensor("wk8", [128, 2, 4 * H], F8, kind="ExternalInput")
    cf32 = nc.dram_tensor("cf32", [128, W32], F32, kind="ExternalInput")
    cf16 = nc.dram_tensor("cf16", [128, 2 + S], F16, kind="ExternalInput")
    hst = nc.dram_tensor("hst", [128, C, 2 * S], F32, kind="ExternalOutput")

    eq = mybir.AluOpType.is_equal
    mult = mybir.AluOpType.mult
    add = mybir.AluOpType.add
    Tanh = mybir.ActivationFunctionType.Tanh
    DR = mybir.MatmulPerfMode.DoubleRow

    NG = -(-C // G)    # number of load groups

    with tile.TileContext(nc) as tc:
        with (
            tc.tile_pool(name="consts", bufs=1) as pc,
            tc.tile_pool(name="loads", bufs=6) as pl,
            tc.tile_pool(name="segp", bufs=3) as psg,
            tc.tile_pool(name="work", bufs=10) as pwk,
            # PSUM: ph 1x2 banks + ga 2x1 + gate 2x2 = 8 banks
            tc.tile_pool(name="pph", bufs=1, space="PSUM") as pph,
            tc.tile_pool(name="pga", bufs=2, space="PSUM") as pga,
            tc.tile_pool(name="pgt", bufs=2, space="PSUM") as pgt,
        ):
            # ---- constants: 3 packed DMAs keep startup short ----
            wk8_sb = pc.tile([128, 2, 4 * H], F8)
            nc.sync.dma_start(out=wk8_sb[:, :, 0 : 2 * H], in_=wk8[:, :, 0 : 2 * H])
            cf16_sb = pc.tile([128, 2 + S], F16)
            cf32_sb = pc.tile([128, W32], F32)
            wp18_sb = wk8_sb[:, :, 0 * H : 1 * H]
            id8_sb = wk8_sb[:, :, 1 * H : 2 * H]
            w28_sb = wk8_sb[:, :, 2 * H : 3 * H]
            w18_sb = wk8_sb[:, :, 3 * H : 4 * H]
            qwh_sb = cf16_sb[:, 0:2]
            iota_at_sb = cf16_sb[:, 2 : 2 + S]
            segc_sb = cf32_sb[:, 0 : C * KT].rearrange("p (c k) -> p c k", c=C)
            bch_sb = cf32_sb[0:S, C * KT : C * KT + H]
            iota_s_sb = cf32_sb[0:S, C * KT + H : C * KT + H + 1]
            rec_sb = cf32_sb[0:S, C * KT + H + 1 : C * KT + H + 1 + C]

            T_x16, T_xt8, T_pf8 = {}, {}, {}
            T_segb = {}
            T_as, T_at, T_ph8, T_g1, T_smt, T_aat = (
                {}, {}, {}, {}, {}, {}
            )
            T_hsg = {}

            def emit_loads(g, part=None):
                c = g * G
                ng = min(G, C - c)
                if part in (None, 0):
                    xt8_t = pl.tile([128, G, 2, TC], F8, tag="xt8")
                    nc.sync.dma_start(
                        out=xt8_t[:, :ng],
                        in_=xt8[c : c + ng].rearrange("c p i t -> p c i t"),
                    )
                    pf8_t = pl.tile([128, G, 2, TC], F8, tag="pf8")
                    nc.sync.dma_start(
                        out=pf8_t[:, :ng],
                        in_=pf8[c : c + ng].rearrange("c p i t -> p c i t"),
                    )
                    for j in range(ng):
                        T_xt8[c + j] = xt8_t[:, j]
                        T_pf8[c + j] = pf8_t[:, j]
                if part in (None, 1):
                    x16_t = pl.tile([128, G, KT, H], F16, tag="x16")
                    nc.sync.dma_start(
                        out=x16_t[:, :ng],
                        in_=x16[c : c + ng].rearrange("c p k h -> p c k h"),
                    )
                    for j in range(ng):
                        T_x16[c + j] = x16_t[:, j]

            def emit_seg(sg):
                c = sg * SG
                n = min(SG, C - c)
                src = seg_row[c]
                segb = psg.tile([S, SG * TC], F16, tag="segb")
                nc.sync.dma_start(
                    out=segb[:, : n * TC],
                    in_=bass.AP(tensor=src.tensor, offset=src.offset,
                                ap=[[0, S], [1, n * TC]]),
                )
                T_segb[sg] = segb

            emit_loads(0, part=0)
            nc.sync.dma_start(out=wk8_sb[:, :, 2 * H : 4 * H],
                              in_=wk8[:, :, 2 * H : 4 * H])
            nc.gpsimd.dma_start(out=cf16_sb, in_=cf16[:])
            nc.gpsimd.dma_start(out=cf32_sb, in_=cf32[:])
            emit_seg(0)
            emit_loads(0, part=1)
            if NG > 1:
                emit_loads(1)

            for it in range(C + 3):
                c0, c1, c2, c3 = it, it - 1, it - 2, it - 3

                # prefetch
                if c0 % G == 0 and c0 // G + 2 < NG:
                    emit_loads(c0 // G + 2)
                if c0 % SG == 0 and c0 // SG + 1 <= (C - 1) // SG:
                    emit_seg(c0 // SG + 1)

                # ---- masks(c0): first on DVE (deps always ready)
                if c0 < C:
                    sg, si = c0 // SG, c0 % SG
                    segb = T_segb[sg]
                    a_s = pwk.tile([S, TC], F16, tag="a_s")
                    nc.vector.tensor_single_scalar(
                        out=a_s, in_=segb[:, si * TC : (si + 1) * TC],
                        scalar=iota_s_sb, op=eq,
                    )
                    a_t = pwk.tile([128, KT, S], F16, tag="a_t")
                    nc.vector.tensor_tensor(
                        out=a_t,
                        in0=bass.AP(tensor=iota_at_sb.tensor, offset=iota_at_sb.offset,
                                    ap=[list(iota_at_sb.ap[0]), [0, KT], [1, S]]),
                        in1=bass.AP(tensor=segc_sb.tensor,
                                    offset=segc_sb.offset + c0 * KT,
                                    ap=[list(segc_sb.ap[0]), [1, KT], [0, S]]),
                        op=eq,
                    )
                    T_as[c0] = a_s
                    T_at[c0] = a_t

                # ---- gate(c2) matmuls + tanh
                if 0 <= c2 < C:
                    g1_2 = T_g1.pop(c2)
                    a_s2 = T_as.pop(c2)
                    ph8_2 = T_ph8.pop(c2)
                    gp = pgt.tile([128, 2 * TC], F32, tag="gate")
                    for h in range(2):
                        dst = gp[:, h * TC : (h + 1) * TC]
                        nc.tensor.matmul(
                            dst, g1_2[:, h * 128 : (h + 1) * 128], a_s2,
                            start=True, stop=False,
                        )
                        nc.tensor.matmul(
                            dst,
                            w28_sb[:, :, h * 128 : (h + 1) * 128],
                            ph8_2.rearrange("p (i t) -> p i t", i=2),
                            start=False, stop=True, perf_mode=DR,
                        )
                    gt2 = pwk.tile([128, 2 * TC], F16, tag="gt")
                    nc.scalar.activation(out=gt2, in_=gp, func=Tanh, scale=0.5)
                    alp = gp[:, 0:KT]
                    for k in range(KT):
                        for h in range(2):
                            nc.tensor.matmul(
                                alp[:, k : k + 1],
                                gt2[:, h * TC + k * 128 : h * TC + (k + 1) * 128],
                                qwh_sb[:, h : h + 1],
                                start=(h == 0), stop=(h == 1),
                            )
                    alpha = pwk.tile([128, KT], F32, tag="alpha")
                    nc.vector.tensor_scalar(
                        out=alpha, in0=alp, scalar1=0.5, scalar2=qbp,
                        op0=mult, op1=add,
                    )
                    a_t2 = T_at.pop(c2)
                    aat = pwk.tile([128, KT, S], F16, tag="aat")
                    nc.vector.tensor_tensor(
                        out=aat,
                        in0=a_t2,
                        in1=bass.AP(tensor=alpha.tensor, offset=alpha.offset,
                                    ap=[list(alpha.ap[0]), [1, KT], [0, S]]),
                        op=mult,
                    )
                    T_aat[c2] = (gp, aat)

                # ---- ph(c0): DR matmuls + tanh -> fp8
                if c0 < C:
                    xt8_0 = T_xt8.pop(c0)
                    pf8_0 = T_pf8.pop(c0)
                    php = pph.tile([128, 2 * TC], F32, tag="ph")
                    for h in range(2):
                        dst = php[:, h * TC : (h + 1) * TC]
                        nc.tensor.matmul(
                            dst, wp18_sb[:, :, h * 128 : (h + 1) * 128], xt8_0,
                            start=True, stop=False, perf_mode=DR,
                        )
                        nc.tensor.matmul(
                            dst, id8_sb[:, :, h * 128 : (h + 1) * 128], pf8_0,
                            start=False, stop=True, perf_mode=DR,
                        )
                    ph8 = pwk.tile([128, 2 * TC], F8, tag="ph8")
                    nc.scalar.activation(out=ph8, in_=php, func=Tanh)
                    T_ph8[c0] = ph8

                # ---- ss(c0): transposed session sums + fp8 copy
                if c0 < C:
                    x16_0 = T_x16[c0]
                    a_t0 = T_at[c0]
                    ga = pga.tile([128, 2 * S + H], F32, tag="ga")
                    ss = ga[:, 0 : 2 * S]
                    for h in range(2):
                        for k in range(KT):
                            nc.tensor.matmul(
                                ss[:, h * S : (h + 1) * S],
                                x16_0[:, k, h * 128 : (h + 1) * 128],
                                a_t0[:, k, :],
                                start=(k == 0), stop=(k == KT - 1),
                            )
                    smt = pwk.tile([128, 2 * S], F8, tag="smt")
                    nc.vector.tensor_copy(out=smt, in_=ss)
                    T_smt[c0] = (smt, ga)

                # ---- g1(c1): DR matmul + scale/bias (late on PE so the
                # smt copy from last iteration has fully drained)
                if 0 <= c1 < C:
                    smt1, ga1 = T_smt.pop(c1)
                    g1p = ga1[0:S, 2 * S : 2 * S + H]
                    nc.tensor.matmul(
                        g1p,
                        smt1.rearrange("p (i s) -> p i s", i=2),
                        w18_sb[:],
                        start=True, stop=True, perf_mode=DR,
                    )
                    g1 = pwk.tile([S, H], F16, tag="g1")
                    nc.vector.scalar_tensor_tensor(
                        out=g1, in0=g1p, scalar=rec_sb[:, c1 : c1 + 1],
                        in1=bch_sb, op0=mult, op1=add,
                    )
                    T_g1[c1] = g1

                # ---- h_s(c3): transposed weighted sums, copy, store
                if 0 <= c3 < C:
                    gb3, aat3 = T_aat.pop(c3)
                    x16_3 = T_x16.pop(c3)
                    hsp = gb3[:, TC : TC + 2 * S]
                    for h in range(2):
                        for k in range(KT):
                            nc.tensor.matmul(
                                hsp[:, h * S : (h + 1) * S],
                                x16_3[:, k, h * 128 : (h + 1) * 128],
                                aat3[:, k, :],
                                start=(k == 0), stop=(k == KT - 1),
                            )
                    grp = c3 // GST
                    if c3 % GST == 0:
                        T_hsg[grp] = [pwk.tile([128, GST, 2 * S], F32, tag="hsg",
                                               name="hsg"), c3]
                    hsg, lo = T_hsg[grp]
                    nc.vector.tensor_copy(out=hsg[:, c3 % GST], in_=hsp)
                    last_grp = grp == (C - 1) // GST
                    end = c3 % GST == GST - 1 or c3 == C - 1
                    if end or (last_grp and c3 % 2 == 1):
                        nc.gpsimd.dma_start(
                            out=hst[:, lo : c3 + 1, :],
                            in_=hsg[:, lo - grp * GST : c3 % GST + 1],
                        )
                        T_hsg[grp][1] = c3 + 1
                        if end:
                            del T_hsg[grp]

    _split_multi_waits(nc)
    return nc


_CACHE = {}


def kernel(hidden, pos_table, W_pos, b_pos, W1, b1, W2, b2, qw, qb,
           seq_len, reverse_pos):
    seq_len_np = np.asarray(seq_len)
    lens, core_chunks, C = _plan(seq_len_np)
    weights, qbp, pw8 = _pack_weights(
        pos_table, W_pos, b_pos, W1, b1, W2, b2, qw, qb
    )
    x16, xt8, pf8, seg_row, seg_col, recip, unpack_idx = _pack_inputs(
        hidden, reverse_pos, pw8, lens, core_chunks, C
    )

    key = (C, qbp)
    if key not in _CACHE:
        _CACHE[key] = _build_bass(C, qbp)
    nc = _CACHE[key]

    CKT = C * KT
    W32 = CKT + H + 1 + C
    in_maps = []
    for core in range(N_CORES):
        cf32 = np.zeros((128, W32), np.float32)
        cf32[:, :CKT] = seg_col[core].reshape(128, CKT)
        cf32[:S, CKT : CKT + H] = weights["bchrow"]
        cf32[:S, CKT + H] = np.arange(S, dtype=np.float32)
        cf32[:S, CKT + H + 1 :] = recip[core]
        m = dict(
            x16=x16[core], xt8=xt8[core], pf8=pf8[core],
            seg_row=seg_row[core], cf32=cf32,
            wk8=weights["wk8"], cf16=weights["cf16"],
        )
        in_maps.append(m)

    import time as _time

    t0 = _time.perf_counter()
    res = run_bass_kernel_spmd(nc, in_maps, core_ids=list(range(N_CORES)))
    kernel._last_run_s = _time.perf_counter() - t0
    # hst: [N, 128, C, 2S] f32 -> h_s[sess, h] with h = 128*half + p
    hs_all = np.stack([res.results[i]["hst"] for i in range(N_CORES)])
    hs_all = hs_all.reshape(N_CORES, 128, C, 2, S)

    out_core, out_chunk, out_local = unpack_idx
    # [sess, half, p] -> [sess, 128*half + p]
    out = hs_all[out_core, :, out_chunk, :, out_local]      # [B, 128, 2]
    out = out.transpose(0, 2, 1).reshape(len(out_core), H)
    return np.ascontiguousarray(out)
